# revision 34
# baseline (speedup 1.0000x reference)
"""Trainium2 Bass kernel for nn_MeshTransformer (S=1024, D=512, H=8, L=2).

Sequence-parallel over 8 NeuronCores: each core computes its 128-query-row
block of attention/FFN; K/V are computed replicated from the (all-gathered)
full x. Feature-major layout (xT [D, S]) so every linear layer uses its
weight matrix directly as the stationary matmul operand. Matmuls run in
bf16 with f32 PSUM accumulation; the residual/LN spine stays f32.

v2 (perf rewrite, evidence from neuron-profile trace of v1):
- distance-bias is added on DVE (gamma_h * dist table built on GpSimd once
  per layer) instead of 64 scaled-identity PE matmuls per layer.
- all PSUM->SBUF casts moved off ScalarE (slow at copies) to VectorE;
  ScalarE only does exp/sqrt.
- x_own spine is a single [128, 512] f32 tile: batched LN / residual ops.
- all weights DMA-prefetched at kernel start in priority order.
- scores/attnV software-pipelined per j-block to keep PE warm (HAM).
- 1/8 attention scale folded into qw on host.
"""
import numpy as np

S, FEAT, D, H, L, DFF, C = 1024, 64, 512, 8, 2, 2048, 10
DB = D // 4
HD = D // H          # 64 head dim
NCORES = 8
SB = S // NCORES     # 128 own-query block
P = 128
NDCH = D // P        # 4
NFCH = DFF // P      # 16
NJCH = S // P        # 8
VW = HD + 1          # 65: head block width in V (data + ones column)
EPS = 1e-5

_nc_cache = {}


def _build(flags):
    import concourse.bacc as bacc
    from concourse import mybir, tile

    dt = mybir.dt
    AF = mybir.ActivationFunctionType
    ALU = mybir.AluOpType
    f32 = dt.float32
    b16 = dt.bfloat16
    f8 = dt.float8e4
    AX = mybir.AxisListType

    nc = bacc.Bacc("TRN2", num_devices=NCORES, target_bir_lowering=False, debug=False)

    def inp(name, shape, dtype=f32):
        return nc.declare_dram_parameter(name, list(shape), dtype, isOutput=False)

    featT_h = inp("featT", [FEAT, S], b16)
    featTo_h = inp("featT_own", [FEAT, SB], b16)
    peT_h = inp("peT", [D, S], b16)
    peTo_h = inp("peT_own", [D, SB])
    Laug_h = inp("Laug", [4, S])
    Raug_h = inp("Raug_own", [4, SB])
    sqcol_h = inp("sqcol", [S, 1])
    gamT_h = inp("gamT", [P, L * H])
    inw_h = inp("in_w", [FEAT, D], b16)
    inb_h = inp("in_b", [D, 1])
    qw_h = inp("qw2", [L * D, D], b16)   # pre-scaled by 1/8 on host
    kw_h = inp("kw2", [L * D, D], b16)
    vw_h = inp("vw2", [L * D, D], b16)
    ow_h = inp("ow2", [L * D, D], b16)
    qb_h = inp("qb2", [L * D, 1])        # pre-scaled by 1/8 on host
    kb_h = inp("kb2", [L * D, 1])
    vb_h = inp("vb2", [L * D, 1])
    ob_h = inp("ob2", [L * D, 1])
    f1w_h = inp("f1w2", [L * D, DFF], b16)
    f2w_h = inp("f2w2", [L * DFF, D], b16)
    f1b_h = inp("f1b2", [L * DFF, 1])
    f2b_h = inp("f2b2", [L * D, 1])
    n1g_h = inp("n1g2", [L * D, 1])
    n1b_h = inp("n1b2", [L * D, 1])
    n2g_h = inp("n2g2", [L * D, 1])
    n2b_h = inp("n2b2", [L * D, 1])
    if not flags["db1b_z"]:
        biasT_h = inp("biasT_own", [L * H * S, SB])

    y_h = nc.declare_dram_parameter("y", [D, 1], f32, isOutput=True)

    with tile.TileContext(nc) as tc:
        with (
            tc.tile_pool(name="const", bufs=1) as cp,
            tc.tile_pool(name="wts", bufs=1) as wp,
            tc.tile_pool(name="act", bufs=1) as ap,
            tc.tile_pool(name="work", bufs=1) as kp,
            tc.tile_pool(name="ps", bufs=1, space="PSUM") as pp,
            tc.tile_pool(name="dram", bufs=1, space="DRAM") as dp,
        ):
            # ---- DMA prefetch, priority order: x0 path first ----
            inw = cp.tile([FEAT, D], b16)
            nc.sync.dma_start(inw[:], inw_h[:, :])
            featT = cp.tile([FEAT, S], b16)
            nc.sync.dma_start(featT[:], featT_h[:, :])
            featTo = cp.tile([FEAT, SB], b16)
            nc.sync.dma_start(featTo[:], featTo_h[:, :])
            Laug = cp.tile([4, S], f32)
            nc.sync.dma_start(Laug[:], Laug_h[:, :])
            Raug = cp.tile([4, SB], f32)
            nc.sync.dma_start(Raug[:], Raug_h[:, :])
            sqc = cp.tile([P, NJCH], f32)
            nc.sync.dma_start(
                sqc[:], sqcol_h[:, :].rearrange("(c p) o -> p (c o)", c=NJCH, p=P))
            gam = cp.tile([P, L * H], f32)
            nc.sync.dma_start(gam[:], gamT_h[:, :])
            peTo = cp.tile([P, NDCH * SB], f32)   # [128, (d, q)]
            for d in range(NDCH):
                nc.sync.dma_start(peTo[:, d * SB:(d + 1) * SB],
                                  peTo_h[d * P:(d + 1) * P, :])
            pe_t = [cp.tile([P, S], b16, name=f"pe{d}") for d in range(NDCH)]
            for d in range(NDCH):
                nc.sync.dma_start(pe_t[d][:], peT_h[d * P:(d + 1) * P, :])
            inb = None
            if not flags["in_b_z"]:
                inb = cp.tile([P, NDCH], f32)
                nc.sync.dma_start(
                    inb[:], inb_h[:, :].rearrange("(c p) o -> p (c o)", c=NDCH, p=P))

            # per-layer weights: all prefetched up front (SBUF resident)
            qw, kw, vw, ow, f1w, f2w = [], [], [], [], [], []
            for l in range(L):
                qw.append([wp.tile([P, D], b16, name=f"qw{l}{d}") for d in range(NDCH)])
                kw.append([wp.tile([P, D], b16, name=f"kw{l}{d}") for d in range(NDCH)])
                vw.append([wp.tile([P, D], b16, name=f"vw{l}{d}") for d in range(NDCH)])
                ow.append([wp.tile([P, D], b16, name=f"ow{l}{d}") for d in range(NDCH)])
                f1w.append([wp.tile([P, DFF], b16, name=f"f1w{l}{d}")
                            for d in range(NDCH)])
                f2w.append([wp.tile([P, D], b16, name=f"f2w{l}{f}")
                            for f in range(NFCH)])
            for l in range(L):
                for d in range(NDCH):
                    r0 = l * D + d * P
                    nc.sync.dma_start(qw[l][d][:], qw_h[r0:r0 + P, :])
                    nc.sync.dma_start(kw[l][d][:], kw_h[r0:r0 + P, :])
                    nc.sync.dma_start(vw[l][d][:], vw_h[r0:r0 + P, :])
                for d in range(NDCH):
                    r0 = l * D + d * P
                    nc.sync.dma_start(ow[l][d][:], ow_h[r0:r0 + P, :])
                    nc.sync.dma_start(f1w[l][d][:], f1w_h[r0:r0 + P, :])
                for f in range(NFCH):
                    r0 = l * DFF + f * P
                    nc.sync.dma_start(f2w[l][f][:], f2w_h[r0:r0 + P, :])

            def lcol(handle, l, nch, name):
                t = cp.tile([P, nch], f32, name=f"{name}{l}")
                nc.sync.dma_start(
                    t[:], handle[l * nch * P:(l + 1) * nch * P, :]
                    .rearrange("(c p) o -> p (c o)", c=nch, p=P))
                return t

            # constants
            ones_colb = cp.tile([P, 1], b16)
            nc.gpsimd.memset(ones_colb[:], 1.0)
            ones_row = cp.tile([1, P], f32)
            nc.gpsimd.memset(ones_row[:], 1.0)
            eps_c = cp.tile([1, 1], f32)
            nc.gpsimd.memset(eps_c[:], EPS)
            eps30 = cp.tile([P, 1], f32)
            nc.gpsimd.memset(eps30[:], 1e-30)
            ident = cp.tile([P, P], f32)
            nc.gpsimd.memset(ident[:], 1.0)
            nc.gpsimd.affine_select(
                ident[:], ident[:], [[1, P]], ALU.is_equal, 0.0,
                base=0, channel_multiplier=-1)

            # zero-padded Q parity tiles: data halves rewritten per layer,
            # zero halves memset once here.
            qTz = [ap.tile([P, NDCH * SB], b16, name=f"qTz{z}") for z in range(2)]
            nc.gpsimd.memset(qTz[0][HD:P, :], 0.0)
            nc.gpsimd.memset(qTz[1][0:HD, :], 0.0)

            # V tiles [128, 8*65] persist across layers; ones columns set once.
            v_nat = [kp.tile([P, H * VW], b16, name=f"v_{j}") for j in range(NJCH)]
            for j in range(NJCH):
                nc.gpsimd.memset(v_nat[j][:, HD:H * VW:VW], 1.0)

            # ---------------- x0 = in-proj + positional enc ----------------
            x_full = []   # 4 tiles [128, 1024] bf16 — layer-input x (transposed)
            for d in range(NDCH):
                xt = kp.tile([P, S], b16, name=f"xf_{d}_0", tag=f"xf{d}")
                for h2 in range(2):
                    ps = pp.tile([P, 512], f32, name=f"ps_x{d}{h2}", tag="mm", bufs=2)
                    nc.tensor.matmul(
                        ps[:], inw[:, d * P:(d + 1) * P],
                        featT[:, h2 * 512:(h2 + 1) * 512], start=True, stop=True)
                    nc.vector.tensor_add(
                        xt[:, h2 * 512:(h2 + 1) * 512], ps[:],
                        pe_t[d][:, h2 * 512:(h2 + 1) * 512])
                if inb is not None:
                    nc.vector.tensor_scalar_add(xt[:], xt[:], inb[:, d:d + 1])
                x_full.append(xt)

            # x_own spine: single [128, (d, q)] f32 tile + bf16 copy
            ps = pp.tile([P, 512], f32, name="ps_x0o", tag="mm", bufs=2)
            for d in range(NDCH):
                nc.tensor.matmul(ps[:, d * SB:(d + 1) * SB],
                                 inw[:, d * P:(d + 1) * P], featTo[:],
                                 start=True, stop=True)
            x_own = kp.tile([P, NDCH * SB], f32, name="xo0", tag="lnb")
            nc.vector.tensor_add(x_own[:], ps[:], peTo[:])
            if inb is not None:
                for d in range(NDCH):
                    nc.vector.tensor_scalar_add(
                        x_own[:, d * SB:(d + 1) * SB],
                        x_own[:, d * SB:(d + 1) * SB], inb[:, d:d + 1])
            x_own_b = kp.tile([P, NDCH * SB], b16, name="xo0b", tag="lnbb")
            nc.vector.tensor_copy(x_own_b[:], x_own[:])

            # ---------------- pairwise distances distT [128, (j, q)] bf16 ----
            # sqrt(x) computed as exp(0.5*ln(x+tiny)) so ScalarE stays on the
            # natural_log_exp table set for the whole kernel (an ACT_TABLE_LOAD
            # to switch sets costs ~2.7us).
            distT = kp.tile([P, NJCH * SB], b16, name="distT")
            for g in range(2):
                ps = pp.tile([P, 512], f32, name=f"ps_d{g}", tag="mm", bufs=2)
                for jj in range(4):
                    j = g * 4 + jj
                    nc.tensor.matmul(ps[:, jj * P:(jj + 1) * P],
                                     Laug[:, j * P:(j + 1) * P], Raug[:],
                                     start=True, stop=True)
                dsq = ap.tile([P, 512], f32, name=f"dsq{g}", tag="dsq", bufs=1)
                for jj in range(4):
                    j = g * 4 + jj
                    nc.vector.tensor_scalar(
                        dsq[:, jj * P:(jj + 1) * P], ps[:, jj * P:(jj + 1) * P],
                        sqc[:, j:j + 1], 0.0, ALU.add, ALU.max)
                nc.scalar.activation(
                    distT[:, g * 512:(g + 1) * 512], dsq[:], AF.Sqrt)

            # gd[l] = gamma_lh * dist, layout [128, (j, h, q)] bf16.
            # Built on GpSimd (SBUF-only) so it overlaps PE work; layer l+1's
            # table is built during layer l (before the collective trigger so
            # it isn't stuck behind it in the GpSimd FIFO).
            def emit_gd(l):
                if flags["db1b_z"]:
                    gdt = kp.tile([P, NJCH * H * SB], b16, name=f"gd{l}", tag="gd")
                    gd_v = gdt[:, :].rearrange("p (j h q) -> p j h q",
                                               j=NJCH, h=H, q=SB)
                    dist_v = distT[:, :].rearrange("p (j q) -> p j q", j=NJCH)
                    for h in range(H):
                        nc.vector.tensor_scalar_mul(
                            gd_v[:, :, h:h + 1, :].squeeze(2), dist_v[:, :, :],
                            gam[:, l * H + h:l * H + h + 1])
                else:
                    gdt = kp.tile([P, NJCH * H * SB], b16, name=f"bt{l}", tag="gd")
                    for j in range(NJCH):
                        for h in range(H):
                            r0 = ((l * H + h) * NJCH + j) * P
                            nc.sync.dma_start(
                                gdt[:, (j * H + h) * SB:(j * H + h + 1) * SB],
                                biasT_h[r0:r0 + P, :])
                return gdt

            # ---------------- layers ----------------
            gd = emit_gd(0)
            for l in range(L):
                qb = None if flags["qb_z"] else lcol(qb_h, l, NDCH, "qb")
                kb = None if flags["kb_z"] else lcol(kb_h, l, NDCH, "kb")
                ob = None if flags["ob_z"] else lcol(ob_h, l, NDCH, "ob")
                f1b = None if flags["f1b_z"] else lcol(f1b_h, l, NFCH, "f1b")
                f2b = None if flags["f2b_z"] else lcol(f2b_h, l, NDCH, "f2b")
                n1g = None if flags["n1g_1"] else lcol(n1g_h, l, NDCH, "n1g")
                n1b = None if flags["n1b_z"] else lcol(n1b_h, l, NDCH, "n1b")
                n2g = None if flags["n2g_1"] else lcol(n2g_h, l, NDCH, "n2g")
                n2b = None if flags["n2b_z"] else lcol(n2b_h, l, NDCH, "n2b")
                vbr = None
                if not flags["vb_z"]:
                    vbr = cp.tile([1, D], f32, name=f"vbr{l}")
                    nc.sync.dma_start(
                        vbr[:], vb_h[l * D:(l + 1) * D, :].rearrange("p o -> o p"))

                # -- Q^T own, as two zero-padded parity tiles (qTz[z] holds
                # head-parity z's 64 channels, other 64 partitions zero) so
                # every scores matmul is a full-128-contraction at base
                # partition 0 (single-matmul groups closing at row-tile
                # (64,0) crash the runtime).
                psq = pp.tile([P, 512], f32, name=f"ps_q{l}", tag="mm", bufs=2)
                for d in range(NDCH):
                    for dk in range(NDCH):
                        nc.tensor.matmul(
                            psq[:, d * SB:(d + 1) * SB],
                            qw[l][dk][:, d * P:(d + 1) * P],
                            x_own_b[:, dk * SB:(dk + 1) * SB],
                            start=(dk == 0), stop=(dk == NDCH - 1))
                if qb is None:
                    nc.vector.tensor_copy(qTz[0][0:HD, :], psq[0:HD, :])
                    nc.vector.tensor_copy(qTz[1][HD:P, :], psq[HD:P, :])
                else:
                    for d in range(NDCH):
                        nc.vector.tensor_scalar_add(
                            qTz[0][0:HD, d * SB:(d + 1) * SB],
                            psq[0:HD, d * SB:(d + 1) * SB], qb[0:HD, d:d + 1])
                        nc.vector.tensor_scalar_add(
                            qTz[1][HD:P, d * SB:(d + 1) * SB],
                            psq[HD:P, d * SB:(d + 1) * SB], qb[HD:P, d:d + 1])

                # -- K^T full [128, 1024] x 4 --
                kT = [ap.tile([P, S], b16, name=f"kT_{l}_{d}", tag=f"kT{d}")
                      for d in range(NDCH)]
                for d in range(NDCH):
                    for h2 in range(2):
                        ps = pp.tile([P, 512], f32, name=f"ps_k{l}{d}{h2}",
                                     tag="mm", bufs=2)
                        for dk in range(NDCH):
                            nc.tensor.matmul(
                                ps[:], kw[l][dk][:, d * P:(d + 1) * P],
                                x_full[dk][:, h2 * 512:(h2 + 1) * 512],
                                start=(dk == 0), stop=(dk == NDCH - 1))
                        if kb is None:
                            nc.vector.tensor_copy(
                                kT[d][:, h2 * 512:(h2 + 1) * 512], ps[:])
                        else:
                            nc.vector.tensor_scalar_add(
                                kT[d][:, h2 * 512:(h2 + 1) * 512], ps[:],
                                kb[:, d:d + 1])

                # -- V natural [j, (h,c)+ones] (full S) --
                for j in range(NJCH):
                    ps = pp.tile([P, D], f32, name=f"ps_v{l}{j}", tag="mm", bufs=2)
                    for dk in range(NDCH):
                        nc.tensor.matmul(
                            ps[:], x_full[dk][:, j * P:(j + 1) * P], vw[l][dk][:],
                            start=(dk == 0), stop=(dk == NDCH - 1 and vbr is None))
                    if vbr is not None:
                        nc.tensor.matmul(ps[:], ones_row[:], vbr[:],
                                         start=False, stop=True)
                    nc.vector.tensor_copy(
                        v_nat[j][:, :].rearrange("p (h c) -> p h c", c=VW)[:, :, 0:HD],
                        ps[:, :].rearrange("p (h c) -> p h c", c=HD))

                # -- attention: per-j pipeline: scores (PE) -> +bias (DVE) ->
                #    exp (ScalarE) -> attnV accumulation (PE) --
                outUa = pp.tile([P, 4 * VW], f32, name=f"ps_outUa{l}",
                                tag="outUa", bufs=1)
                outUb = pp.tile([P, 4 * VW], f32, name=f"ps_outUb{l}",
                                tag="outUb", bufs=1)
                eTas = []
                for j in range(NJCH):
                    scA = pp.tile([P, S], f32, name=f"ps_scA{l}{j}",
                                  tag="scA", bufs=2)
                    for h in range(H):
                        t2 = h // 2
                        nc.tensor.matmul(
                            scA[:, h * P:(h + 1) * P],
                            kT[t2][:, j * P:(j + 1) * P],
                            qTz[h % 2][:, t2 * SB:(t2 + 1) * SB],
                            start=True, stop=True)
                    eIn = ap.tile([P, S], b16, name=f"eIn{l}{j}", tag="eIn", bufs=2)
                    eTa = ap.tile([P, S], b16, name=f"eTa{l}{j}", tag="eTA", bufs=8)
                    for hf in range(2):
                        nc.vector.tensor_add(
                            eIn[:, hf * 512:(hf + 1) * 512],
                            scA[:, hf * 512:(hf + 1) * 512],
                            gd[:, j * H * SB + hf * 512:j * H * SB + (hf + 1) * 512])
                        nc.scalar.activation(
                            eTa[:, hf * 512:(hf + 1) * 512],
                            eIn[:, hf * 512:(hf + 1) * 512], AF.Exp)
                    eTas.append(eTa)
                # attnV: one PSUM accumulation group open per bank at a time
                # (hardware zero-region constraint); heads h and h+4 live in
                # different banks so their groups can interleave.
                for hp in range(4):
                    for j in range(NJCH):
                        nc.tensor.matmul(
                            outUa[:, hp * VW:(hp + 1) * VW],
                            eTas[j][:, hp * P:(hp + 1) * P],
                            v_nat[j][:, hp * VW:(hp + 1) * VW],
                            start=(j == 0), stop=(j == NJCH - 1))
                        h2_ = hp + 4
                        nc.tensor.matmul(
                            outUb[:, hp * VW:(hp + 1) * VW],
                            eTas[j][:, h2_ * P:(h2_ + 1) * P],
                            v_nat[j][:, h2_ * VW:(h2_ + 1) * VW],
                            start=(j == 0), stop=(j == NJCH - 1))

                # build next layer's bias table now: GpSimd is idle during
                # FFN, and it must precede the collective in GpSimd's FIFO.
                if l + 1 < L:
                    gd = emit_gd(l + 1)

                # normalize: outS[q, (h, c)] = outU[:, h-block] / norm_h[q]
                rva = ap.tile([P, 4], f32, name=f"rva{l}", tag="rva")
                nc.vector.reciprocal(rva[:], outUa[:, HD:4 * VW:VW])
                rvb = ap.tile([P, 4], f32, name=f"rvb{l}", tag="rvb")
                nc.vector.reciprocal(rvb[:], outUb[:, HD:4 * VW:VW])
                outS = ap.tile([P, D], f32, name=f"outS{l}", tag="outS", bufs=1)
                for h in range(H):
                    oU, rv = (outUa, rva) if h < 4 else (outUb, rvb)
                    hb = (h % 4) * VW
                    nc.scalar.activation(
                        outS[:, h * HD:(h + 1) * HD],
                        oU[:, hb:hb + HD], AF.Copy,
                        scale=rv[:, h % 4:h % 4 + 1])

                # transpose attn output to [c, q] for the O-projection
                pst = pp.tile([P, 512], f32, name=f"ps_tr{l}", tag="mm", bufs=2)
                for c in range(NDCH):
                    nc.tensor.transpose(
                        pst[:, c * P:(c + 1) * P],
                        outS[:, c * P:(c + 1) * P], ident[:])
                outT = ap.tile([P, NDCH * SB], b16, name=f"outT{l}", tag="outT")
                nc.vector.tensor_copy(outT[:], pst[:])

                # -- O-projection + residual (batched into spine) --
                pso = pp.tile([P, 512], f32, name=f"ps_o{l}", tag="mm", bufs=2)
                for d in range(NDCH):
                    for c in range(NDCH):
                        nc.tensor.matmul(
                            pso[:, d * SB:(d + 1) * SB],
                            ow[l][c][:, d * P:(d + 1) * P],
                            outT[:, c * SB:(c + 1) * SB],
                            start=(c == 0), stop=(c == NDCH - 1))
                xres = kp.tile([P, NDCH * SB], f32, name=f"xr1_{l}", tag="xr")
                nc.vector.tensor_add(xres[:], pso[:], x_own[:])
                if ob is not None:
                    for d in range(NDCH):
                        nc.vector.tensor_scalar_add(
                            xres[:, d * SB:(d + 1) * SB],
                            xres[:, d * SB:(d + 1) * SB], ob[:, d:d + 1])

                def layernorm(xin, g, b, nm):
                    # xin: [128, (d, q)] f32 spine tile
                    xb = ap.tile([P, NDCH * SB], b16, name=f"lnxb{nm}",
                                 tag="lnxb", bufs=1)
                    nc.vector.tensor_copy(xb[:], xin[:])
                    sq = ap.tile([P, NDCH * SB], b16, name=f"sq{nm}",
                                 tag="lnsq", bufs=1)
                    nc.vector.tensor_mul(sq[:], xb[:], xb[:])
                    srs = pp.tile([1, 2 * P], f32, name=f"ps_srs{nm}",
                                  tag="mm", bufs=2)
                    for d in range(NDCH):
                        nc.tensor.matmul(srs[:, 0:P], ones_colb[:],
                                         xb[:, d * SB:(d + 1) * SB],
                                         start=(d == 0), stop=(d == NDCH - 1))
                    for d in range(NDCH):
                        nc.tensor.matmul(srs[:, P:2 * P], ones_colb[:],
                                         sq[:, d * SB:(d + 1) * SB],
                                         start=(d == 0), stop=(d == NDCH - 1))
                    mu_em = ap.tile([1, 2 * P], f32, name=f"mue{nm}",
                                    tag="lnrow", bufs=4)
                    nc.vector.tensor_scalar_mul(mu_em[:], srs[:], 1.0 / D)
                    mu = mu_em[:, 0:P]
                    mu2 = ap.tile([1, P], f32, name=f"mu2{nm}", tag="lnrow", bufs=4)
                    nc.vector.tensor_mul(mu2[:], mu, mu)
                    var = ap.tile([1, P], f32, name=f"var{nm}", tag="lnrow", bufs=4)
                    nc.vector.tensor_sub(var[:], mu_em[:, P:2 * P], mu2[:])
                    # rstd = rsqrt(var+eps) via DVE Newton iteration (int
                    # bit-trick seed); avoids ScalarE table switches between
                    # sqrt and exp sets (~2.7us per ACT_TABLE_LOAD).
                    vv = ap.tile([1, P], f32, name=f"vv{nm}", tag="lnrow", bufs=4)
                    nc.vector.tensor_scalar_add(vv[:], var[:], EPS)
                    sh = ap.tile([1, P], dt.int32, name=f"sh{nm}", tag="lnrow", bufs=4)
                    nc.vector.tensor_scalar(
                        sh[:], vv[:].bitcast(dt.int32), 1, None,
                        ALU.logical_shift_right)
                    y = ap.tile([1, P], f32, name=f"y{nm}", tag="lnrow", bufs=4)
                    nc.vector.tensor_scalar(
                        y[:].bitcast(dt.int32), sh[:], -1, 0x5F3759DF,
                        ALU.mult, ALU.add)
                    t1 = ap.tile([1, P], f32, name=f"t1{nm}", tag="lnrow", bufs=4)
                    t2 = ap.tile([1, P], f32, name=f"t2{nm}", tag="lnrow", bufs=4)
                    for _ in range(2):
                        nc.vector.tensor_mul(t1[:], y[:], y[:])
                        nc.vector.tensor_mul(t2[:], vv[:], t1[:])
                        nc.vector.tensor_scalar(
                            t2[:], t2[:], -0.5, 1.5, ALU.mult, ALU.add)
                        nc.vector.tensor_mul(y[:], y[:], t2[:])
                    rstd = y
                    # broadcast mu/rstd rows to [128, 128] via PE
                    mub = pp.tile([P, P], f32, name=f"ps_mub{nm}", tag="mm", bufs=2)
                    nc.tensor.matmul(mub[:], ones_row[:], mu, start=True, stop=True)
                    rsb = pp.tile([P, P], f32, name=f"ps_rsb{nm}", tag="mm", bufs=2)
                    nc.tensor.matmul(rsb[:], ones_row[:], rstd[:], start=True, stop=True)
                    mub_b = mub[:, :].unsqueeze(1).to_broadcast([P, NDCH, SB])
                    rsb_b = rsb[:, :].unsqueeze(1).to_broadcast([P, NDCH, SB])
                    xin_v = xin[:, :].rearrange("p (d q) -> p d q", d=NDCH)
                    t = ap.tile([P, NDCH * SB], f32, name=f"lnt{nm}", tag="lntmp",
                                bufs=1)
                    t_v = t[:, :].rearrange("p (d q) -> p d q", d=NDCH)
                    nc.vector.tensor_sub(t_v[:, :, :], xin_v[:, :, :], mub_b)
                    o = kp.tile([P, NDCH * SB], f32, name=f"ln{nm}", tag=f"ln{nm[0]}")
                    o_v = o[:, :].rearrange("p (d q) -> p d q", d=NDCH)
                    nc.vector.tensor_mul(o_v[:, :, :], t_v[:, :, :], rsb_b)
                    if g is not None or b is not None:
                        for d in range(NDCH):
                            gcol = g[:, d:d + 1] if g is not None else 1.0
                            bcol = b[:, d:d + 1] if b is not None else 0.0
                            nc.vector.tensor_scalar(
                                o[:, d * SB:(d + 1) * SB],
                                o[:, d * SB:(d + 1) * SB],
                                gcol, bcol, ALU.mult, ALU.add)
                    ob_ = kp.tile([P, NDCH * SB], b16, name=f"lnb{nm}",
                                  tag=f"lnb{nm[0]}")
                    nc.vector.tensor_copy(ob_[:], o[:])
                    return o, ob_

                x_ln, x_ln_b = layernorm(xres, n1g, n1b, f"a{l}")

                # -- FFN --
                h1 = [ap.tile([P, 4 * SB], b16, name=f"h1_{l}_{g}", tag=f"h1{g}")
                      for g in range(4)]
                for g in range(4):
                    ps = pp.tile([P, 512], f32, name=f"ps_f1{l}{g}", tag="mm",
                                 bufs=2)
                    for ff in range(4):
                        f = g * 4 + ff
                        for dk in range(NDCH):
                            nc.tensor.matmul(
                                ps[:, ff * SB:(ff + 1) * SB],
                                f1w[l][dk][:, f * P:(f + 1) * P],
                                x_ln_b[:, dk * SB:(dk + 1) * SB],
                                start=(dk == 0), stop=(dk == NDCH - 1))
                    if f1b is None:
                        nc.vector.tensor_scalar_max(h1[g][:], ps[:], 0.0)
                    else:
                        for ff in range(4):
                            f = g * 4 + ff
                            nc.vector.tensor_scalar(
                                h1[g][:, ff * SB:(ff + 1) * SB],
                                ps[:, ff * SB:(ff + 1) * SB],
                                f1b[:, f:f + 1], 0.0, ALU.add, ALU.max)
                h2n = pp.tile([P, D], f32, name=f"ps_h2n{l}", tag="scA", bufs=2)
                for f in range(NFCH):
                    nc.tensor.matmul(h2n[:], h1[f // 4][:, (f % 4) * SB:(f % 4 + 1) * SB],
                                     f2w[l][f][:],
                                     start=(f == 0), stop=(f == NFCH - 1))
                h2s = ap.tile([P, D], f32, name=f"h2s{l}", tag="h2s", bufs=1)
                nc.vector.tensor_copy(h2s[:], h2n[:])
                ps2 = pp.tile([P, 512], f32, name=f"ps_h2t{l}", tag="mm", bufs=2)
                for d in range(NDCH):
                    nc.tensor.transpose(ps2[:, d * P:(d + 1) * P],
                                        h2s[:, d * P:(d + 1) * P], ident[:])
                xres2 = kp.tile([P, NDCH * SB], f32, name=f"xr2_{l}", tag="xr")
                nc.vector.tensor_add(xres2[:], ps2[:], x_ln[:])
                if f2b is not None:
                    for d in range(NDCH):
                        nc.vector.tensor_scalar_add(
                            xres2[:, d * SB:(d + 1) * SB],
                            xres2[:, d * SB:(d + 1) * SB], f2b[:, d:d + 1])

                x_own, x_own_b = layernorm(xres2, n2g, n2b, f"b{l}")

                # -- all-gather x (fp8) for next layer's K/V: halves the
                # collective payload; fp8 moving x bf16 stationary matmuls.
                if l + 1 < L:
                    x_own_8 = ap.tile([P, NDCH * SB], f8, name=f"xo8_{l}",
                                      tag="xo8", bufs=1)
                    nc.vector.tensor_copy(x_own_8[:], x_own[:])
                    xo_d = dp.tile([D, SB], f8, name=f"xo_dram{l}")
                    for d in range(NDCH):
                        nc.sync.dma_start(xo_d[d * P:(d + 1) * P, :],
                                          x_own_8[:, d * SB:(d + 1) * SB])
                    xg_d = dp.tile([NCORES * D, SB], f8, name=f"xg_dram{l}",
                                   addr_space="Shared")
                    nc.gpsimd.collective_compute(
                        "AllGather", mybir.AluOpType.bypass,
                        replica_groups=[list(range(NCORES))],
                        ins=[xo_d[:].opt()], outs=[xg_d[:].opt()])
                    x_full = []
                    for d in range(NDCH):
                        xt = kp.tile([P, S], f8, name=f"xf8_{d}_{l + 1}",
                                     tag=f"xf{d}")
                        for r in range(NCORES):
                            r0 = r * D + d * P
                            nc.sync.dma_start(
                                xt[:, r * SB:(r + 1) * SB], xg_d[r0:r0 + P, :])
                        x_full.append(xt)

            # ------------- per-core partial pool output (head on host) -------
            for d in range(NDCH):
                red = ap.tile([P, 1], f32, name=f"red{d}", tag="red", bufs=4)
                nc.vector.reduce_sum(red[:], x_own[:, d * SB:(d + 1) * SB], axis=AX.X)
                nc.sync.dma_start(y_h[d * P:(d + 1) * P, :], red[:])

    nc.compile()
    return nc


def _prep(inputs):
    """Host-side input prep: transposes, positional encoding, bias collapse."""
    import ml_dtypes
    f32 = np.float32
    bf16 = ml_dtypes.bfloat16
    pos = np.asarray(inputs["positions"], f32)          # [S, 3]
    feat = np.asarray(inputs["features"], f32)          # [S, FEAT]
    fb = np.asarray(inputs["freq_bands"], f32)          # [NFREQ]

    enc = []
    for i in range(3):
        cs = pos[:, i:i + 1] * fb[None, :]
        enc.append(np.sin(cs, dtype=f32))
        enc.append(np.cos(cs, dtype=f32))
    pe = np.concatenate(enc, axis=-1).astype(f32)
    if pe.shape[1] < D:
        pe = np.pad(pe, ((0, 0), (0, D - pe.shape[1])))
    peT = np.ascontiguousarray(pe.T)                    # [D, S]

    featT = np.ascontiguousarray(feat.T)                # [FEAT, S]
    posT = np.ascontiguousarray(pos.T)                  # [3, S]
    sq = (pos * pos).sum(1).astype(f32)                 # [S]
    Laug = np.concatenate([-2.0 * posT, np.ones((1, S), f32)], 0)
    Raug = np.concatenate([posT, sq[None, :]], 0)

    db1w = np.asarray(inputs["db1w"], f32)
    db1b = np.asarray(inputs["db1b"], f32)
    db2w = np.asarray(inputs["db2w"], f32)
    db1b_z = bool(np.all(db1b == 0))
    gam = np.zeros((L, H), f32)
    biasT_own = None
    if db1b_z:
        for l in range(L):
            gam[l] = np.maximum(db1w[l, 0], 0.0) @ db2w[l]
    else:
        diff = pos[:, None, :] - pos[None, :, :]
        sqm = np.sum(diff * diff, axis=-1)
        dist = np.sqrt(np.where(sqm > 0, sqm, 1.0)).astype(f32) * (sqm > 0)
        biasT_own = np.zeros((NCORES, L * H * S, SB), f32)
        for l in range(L):
            hbl = np.maximum(dist[:, :, None] * db1w[l, 0][None, None, :]
                             + db1b[l][None, None, :], 0.0).astype(f32)
            bl = np.einsum("ijc,ch->hij", hbl, db2w[l]).astype(f32)
            for c in range(NCORES):
                blk = bl[:, c * SB:(c + 1) * SB, :]
                biasT_own[c, l * H * S:(l + 1) * H * S, :] = (
                    blk.transpose(0, 2, 1).reshape(H * S, SB))
    gamT = np.broadcast_to(gam.reshape(1, L * H), (P, L * H)).copy()

    def col(x):
        return np.ascontiguousarray(np.asarray(x, f32).reshape(-1, 1))

    common = {
        "featT": featT.astype(bf16),
        "peT": peT.astype(bf16),
        "Laug": Laug,
        "sqcol": col(sq),
        "gamT": gamT,
        "in_w": np.asarray(inputs["in_w"], f32).astype(bf16),
        "in_b": col(inputs["in_b"]),
        "qw2": (np.asarray(inputs["qw"], f32).reshape(L * D, D) * 0.125).astype(bf16),
        "kw2": np.asarray(inputs["kw"], f32).reshape(L * D, D).astype(bf16),
        "vw2": np.asarray(inputs["vw"], f32).reshape(L * D, D).astype(bf16),
        "ow2": np.asarray(inputs["ow"], f32).reshape(L * D, D).astype(bf16),
        "qb2": col(np.asarray(inputs["qb"], f32) * 0.125),
        "kb2": col(inputs["kb"]),
        "vb2": col(inputs["vb"]),
        "ob2": col(inputs["ob"]),
        "f1w2": np.asarray(inputs["f1w"], f32).reshape(L * D, DFF).astype(bf16),
        "f2w2": np.asarray(inputs["f2w"], f32).reshape(L * DFF, D).astype(bf16),
        "f1b2": col(inputs["f1b"]),
        "f2b2": col(inputs["f2b"]),
        "n1g2": col(inputs["n1g"]),
        "n1b2": col(inputs["n1b"]),
        "n2g2": col(inputs["n2g"]),
        "n2b2": col(inputs["n2b"]),
    }
    flags = {
        "in_b_z": bool(np.all(common["in_b"] == 0)),
        "qb_z": bool(np.all(common["qb2"] == 0)),
        "kb_z": bool(np.all(common["kb2"] == 0)),
        "vb_z": bool(np.all(common["vb2"] == 0)),
        "ob_z": bool(np.all(common["ob2"] == 0)),
        "f1b_z": bool(np.all(common["f1b2"] == 0)),
        "f2b_z": bool(np.all(common["f2b2"] == 0)),
        "n1g_1": bool(np.all(common["n1g2"] == 1)),
        "n1b_z": bool(np.all(common["n1b2"] == 0)),
        "n2g_1": bool(np.all(common["n2g2"] == 1)),
        "n2b_z": bool(np.all(common["n2b2"] == 0)),
        "db1b_z": db1b_z,
    }
    in_maps = []
    for c in range(NCORES):
        m = dict(common)
        m["featT_own"] = np.ascontiguousarray(
            featT[:, c * SB:(c + 1) * SB]).astype(bf16)
        m["peT_own"] = np.ascontiguousarray(peT[:, c * SB:(c + 1) * SB])
        m["Raug_own"] = np.ascontiguousarray(Raug[:, c * SB:(c + 1) * SB])
        if biasT_own is not None:
            m["biasT_own"] = biasT_own[c]
        in_maps.append(m)
    return flags, in_maps


def get_nc_and_inmaps(inputs):
    flags, in_maps = _prep(inputs)
    key = tuple(sorted(flags.items()))
    if key not in _nc_cache:
        _nc_cache[key] = _build(flags)
    return _nc_cache[key], in_maps


def finish_output(res, inputs):
    f32 = np.float32
    pooled = np.zeros((D,), f32)
    for c in range(NCORES):
        pooled += np.asarray(res.results[c]["y"], f32).reshape(D)
    pooled /= S
    z = np.maximum(pooled @ np.asarray(inputs["c1w"], f32)
                   + np.asarray(inputs["c1b"], f32), 0.0)
    y = z @ np.asarray(inputs["c2w"], f32) + np.asarray(inputs["c2b"], f32)
    return y.reshape(1, C).astype(f32)


def kernel(**inputs) -> np.ndarray:
    from concourse import bass_utils
    nc, in_maps = get_nc_and_inmaps(inputs)
    res = bass_utils.run_bass_kernel_spmd(
        nc, in_maps, core_ids=list(range(NCORES)))
    return finish_output(res, inputs)


if __name__ == "__main__":
    import jax
    cpu = jax.devices("cpu")[0]
    with jax.default_device(cpu):
        import reference
        inputs = {k: np.asarray(jax.device_put(np.asarray(v), cpu))
                  for k, v in reference.setup_inputs().items()}
        exp = np.asarray(reference.reference(**inputs))
    out = kernel(**inputs)
    err = np.abs(out - exp).max() / (np.abs(exp).max() + 1e-12)
    print("out:", out)
    print("exp:", exp)
    print("rel err:", err)
